# revision 3
# baseline (speedup 1.0000x reference)
"""TGCN (GCNConv + GRUCell) Bass kernel for 8 TRN2 NeuronCores — v2.

vs baseline: fp16 pair-gather (512B descriptors, idx = src>>1 so no src-half
split), LPT node rebalancing to equalize per-(core,block) edge counts, self
loops via contiguous DMA of host-prescaled dinv^2*x rows (identity-matrix
matmul, start=True per block), fp16 matmuls/S-tiles/GRU throughout.
"""
import sys

sys.path.insert(0, '/opt/trn_rl_repo')

import heapq
import numpy as np

N = 50000
E_IN = 800000
IN_C = 128
HID = 64
CORES = 8
P = 128
BLOCKS = 49                 # blocks of 128 dst per core
SHARDP = BLOCKS * P         # 6272 padded shard width (bins may exceed 6250)
W = 16                      # narrow one-hot window width
GS = [2, 3] + [5] * 8 + [3, 1]  # blocks per gather group (sum = 49)
PAIR_LAG = 4                # extra blocks required beyond a pair's own range
NCH = 13                    # 512-wide phase-2 chunks (12x512 + 128)
NPAIR = 7

last_nc = None

_splitwait_uid = [0]


def _split_sync_waits(nc, limit=1):
    """Walrus rejects >1 semaphore wait per instruction; move extras onto
    same-engine NoOp carriers inserted immediately before."""
    import concourse.mybir as mybir
    n_fixed = 0
    for f in nc.m.functions:
        for bb in f.blocks:
            insts = bb.instructions
            i = 0
            while i < len(insts):
                inst = insts[i]
                si = inst.sync_info
                if si is not None and si.on_wait is not None and len(si.on_wait) > limit:
                    waits = list(si.on_wait)
                    pre = []
                    while len(waits) > limit:
                        chunk, waits = waits[:limit], waits[limit:]
                        _splitwait_uid[0] += 1
                        pre.append(mybir.InstNoOp(
                            name=f"bass_splitwait_{_splitwait_uid[0]}",
                            engine=inst.engine,
                            sync_info=mybir.SyncInfo(on_wait=chunk, on_update=[]),
                        ))
                    si.on_wait = waits
                    for j, nd in enumerate(pre):
                        insts.insert(i + j, nd)
                    i += len(pre)
                    n_fixed += 1
                i += 1
    return n_fixed


class Plan:
    pass


def _balance_nodes(edeg):
    """LPT: assign N nodes into CORES*BLOCKS bins of exactly 128 nodes,
    balancing per-bin edge sums. Returns bin id per node and slot order."""
    NB = CORES * BLOCKS
    order = np.argsort(-edeg, kind='stable')
    heap = [(0, b) for b in range(NB)]
    heapq.heapify(heap)
    cnt = np.zeros(NB, np.int64)
    assign = np.empty(N, np.int64)
    dib = np.empty(N, np.int64)
    for v in order:
        while True:
            s, b = heapq.heappop(heap)
            if cnt[b] < P:
                break
        assign[v] = b
        dib[v] = cnt[b]
        cnt[b] += 1
        if cnt[b] < P:
            heapq.heappush(heap, (s + int(edeg[v]), b))
    return assign, dib


def _host_prep(x, edge_index, edge_weight):
    src = np.asarray(edge_index[0], dtype=np.int64)
    dst = np.asarray(edge_index[1], dtype=np.int64)
    w = np.asarray(edge_weight, dtype=np.float32)

    deg = (np.bincount(dst, weights=w.astype(np.float64), minlength=N)
           .astype(np.float32) + np.float32(1.0))
    dinv = (np.float32(1.0) / np.sqrt(deg)).astype(np.float32)
    norm = (dinv[src] * w * dinv[dst]).astype(np.float32)

    edeg = np.bincount(dst, minlength=N)
    abin, adib = _balance_nodes(edeg)
    acore = abin // BLOCKS
    ablk = abin % BLOCKS
    apos = ablk * P + adib                        # position within core shard
    # node_at[c, pos] = node (or -1)
    node_at = np.full((CORES, SHARDP), -1, np.int64)
    node_at[acore, apos] = np.arange(N)

    ec = acore[dst]
    eb = ablk[dst]
    edib = adib[dst]
    parity = (src & 1).astype(np.int64)
    idxp = (src >> 1).astype(np.int16)

    key = ec * BLOCKS + eb
    order = np.lexsort((edib, key))
    s_key = key[order]
    s_idx = idxp[order]
    s_dib = edib[order]
    s_norm = norm[order]
    s_par = parity[order]

    counts = np.bincount(s_key, minlength=CORES * BLOCKS).reshape(CORES, BLOCKS)
    Tb = (counts.max(axis=0) + P - 1) // P        # tiles per block
    PB = Tb * P
    slot_off = np.zeros(BLOCKS, np.int64)
    slot_off[1:] = np.cumsum(PB)[:-1]
    TOT = int(PB.sum())
    Ttot = TOT // P

    grp_start = np.zeros(CORES * BLOCKS + 1, np.int64)
    np.cumsum(np.bincount(s_key, minlength=CORES * BLOCKS), out=grp_start[1:])
    rank = np.arange(len(s_key)) - grp_start[s_key]
    b_of = s_key % BLOCKS
    c_of = s_key // BLOCKS
    slot = slot_off[b_of] + rank

    idx_flat = np.zeros((CORES, TOT), np.int16)
    dib_flat = np.full((CORES, TOT), -1, np.int64)
    norme = np.zeros((CORES, TOT), np.float32)
    normo = np.zeros((CORES, TOT), np.float32)
    idx_flat[c_of, slot] = s_idx
    dib_flat[c_of, slot] = s_dib
    norme[c_of, slot] = np.where(s_par == 0, s_norm, 0.0)
    normo[c_of, slot] = np.where(s_par == 1, s_norm, 0.0)

    # per-tile dst range union over cores
    dib_t = dib_flat.reshape(CORES, Ttot, P)
    val_t = dib_t >= 0
    tmin = np.where(val_t, dib_t, 10**6).min(axis=(0, 2))
    tmax = np.where(val_t, dib_t, -1).max(axis=(0, 2))
    empty = tmax < 0
    tmin[empty] = 0
    tmax[empty] = 0
    tile_full = (tmax - tmin) >= W
    w0 = np.minimum(tmin, P - W)
    w0[tile_full] = 0
    full_idx = np.cumsum(tile_full) - 1
    NFULL = int(tile_full.sum())

    w0_slot = w0[np.arange(TOT) // P]
    dstn = (dib_flat - w0_slot[None, :]).astype(np.float32)
    dstn[dib_flat < 0] = -1.0
    dibF = dib_flat.astype(np.float32)
    dibF[dib_flat < 0] = -1.0

    idx_w = np.ascontiguousarray(
        np.tile(idx_flat.reshape(CORES, -1, 16).transpose(0, 2, 1), (1, 8, 1)))
    dstnT = np.ascontiguousarray(
        dstn.reshape(CORES, Ttot, P).transpose(0, 2, 1)).astype(np.float16)
    normeT = np.ascontiguousarray(
        norme.reshape(CORES, Ttot, P).transpose(0, 2, 1)).astype(np.float16)
    normoT = np.ascontiguousarray(
        normo.reshape(CORES, Ttot, P).transpose(0, 2, 1)).astype(np.float16)
    ft = np.flatnonzero(tile_full)
    dstF = np.ascontiguousarray(
        dibF.reshape(CORES, Ttot, P).transpose(0, 2, 1)[:, :, ft]).astype(np.float16)
    normFe = np.ascontiguousarray(
        norme.reshape(CORES, Ttot, P).transpose(0, 2, 1)[:, :, ft]).astype(np.float16)
    normFo = np.ascontiguousarray(
        normo.reshape(CORES, Ttot, P).transpose(0, 2, 1)[:, :, ft]).astype(np.float16)
    if NFULL == 0:
        z = np.full((CORES, P, 1), -1.0, np.float16)
        dstF, normFe, normFo = z, np.zeros_like(z), np.zeros_like(z)

    pl = Plan()
    pl.Tb, pl.slot_off, pl.Ttot, pl.TOT = Tb, slot_off, Ttot, TOT
    pl.tile_full, pl.w0, pl.full_idx, pl.NFULL = tile_full, w0, full_idx, NFULL
    pl.node_at = node_at
    return pl, idx_w, dstnT, normeT, normoT, dstF, normFe, normFo


def _build_program(pl, hzero=True, skip=()):
    import concourse.bass as bass
    import concourse.tile as tile
    import concourse.mybir as mybir
    from concourse import library_config

    Tb, slot_off, Ttot = pl.Tb, pl.slot_off, pl.Ttot
    tile_full, w0, full_idx, NFULL = pl.tile_full, pl.w0, pl.full_idx, pl.NFULL
    full_pos = np.flatnonzero(tile_full)
    HZERO = bool(hzero)

    f16 = mybir.dt.float16
    f32 = mybir.dt.float32
    PH = NPAIR * 512                      # 3584 packed phase-2 columns

    # group structure: block ranges + tile ranges + full-tile ranges
    gblocks, b0 = [], 0
    for gs in GS:
        gblocks.append(list(range(b0, b0 + gs)))
        b0 += gs
    gt0 = [int(slot_off[g[0]]) // P for g in gblocks]          # first tile
    gnt = [int(sum(Tb[b] for b in g)) for g in gblocks]        # tiles in group
    gf0 = [int(np.searchsorted(full_pos, t)) for t in gt0]
    gnf = [int(tile_full[gt0[i]:gt0[i] + gnt[i]].sum()) for i in range(len(GS))]
    Tmax = max(gnt)
    NFmax = max(max(gnf), 1)

    nc = bass.Bass("TRN2", target_bir_lowering=False, debug=False,
                   num_devices=CORES)

    x16_d = nc.dram_tensor("x16", [N // 2, 2 * IN_C], f16, kind="ExternalInput")
    idx_d = nc.dram_tensor("idx", [P, pl.TOT // 16], mybir.dt.int16, kind="ExternalInput")
    dstn_d = nc.dram_tensor("dstn", [P, Ttot], f16, kind="ExternalInput")
    norme_d = nc.dram_tensor("norme", [P, Ttot], f16, kind="ExternalInput")
    normo_d = nc.dram_tensor("normo", [P, Ttot], f16, kind="ExternalInput")
    dstf_d = nc.dram_tensor("dstf", [P, max(NFULL, 1)], f16, kind="ExternalInput")
    normfe_d = nc.dram_tensor("normfe", [P, max(NFULL, 1)], f16, kind="ExternalInput")
    normfo_d = nc.dram_tensor("normfo", [P, max(NFULL, 1)], f16, kind="ExternalInput")
    selfx_d = nc.dram_tensor("selfx", [P, SHARDP], f16, kind="ExternalInput")
    ident_d = nc.dram_tensor("ident", [P, P], f16, kind="ExternalInput")
    iota_d = nc.dram_tensor("iota", [P, P], f16, kind="ExternalInput")
    wgcnT_d = nc.dram_tensor("wgcnT", [IN_C, HID], f16, kind="ExternalInput")
    wihT_d = nc.dram_tensor("wihT", [P, 3 * HID], f16, kind="ExternalInput")
    whhT_d = nc.dram_tensor("whhT", [P, 3 * HID], f16, kind="ExternalInput")
    br_d = nc.dram_tensor("br", [P, 1], f32, kind="ExternalInput")
    bz_d = nc.dram_tensor("bz", [P, 1], f32, kind="ExternalInput")
    bihn_d = nc.dram_tensor("bihn", [P, 1], f32, kind="ExternalInput")
    bhhn_d = nc.dram_tensor("bhhn", [P, 1], f32, kind="ExternalInput")
    hmemT_d = nc.dram_tensor("hmemT", [HID, SHARDP], f16, kind="ExternalInput")
    out_d = nc.dram_tensor("outT", [HID, SHARDP], f16, kind="ExternalOutput")

    with tile.TileContext(nc, trace_sim=False) as tc:
        nc.gpsimd.load_library(library_config.mlp)
        with (
            tc.tile_pool(name="const", bufs=1) as cpool,
            tc.tile_pool(name="agg", bufs=1) as apool,
            tc.tile_pool(name="g", bufs=2) as gpool,
            tc.tile_pool(name="s", bufs=2) as spool,
            tc.tile_pool(name="sf", bufs=2) as sfpool,
            tc.tile_pool(name="ps1", bufs=3, space="PSUM") as ppool,
            tc.tile_pool(name="p2", bufs=2) as sb2,
            tc.tile_pool(name="ps2g", bufs=2, space="PSUM") as pp2g,
            tc.tile_pool(name="ps2", bufs=1, space="PSUM") as pp2,
        ):
            idx_t = cpool.tile([P, pl.TOT // 16], mybir.dt.int16)
            dstn_t = cpool.tile([P, Ttot], f16)
            norme_t = cpool.tile([P, Ttot], f16)
            normo_t = cpool.tile([P, Ttot], f16)
            dstf_t = cpool.tile([P, max(NFULL, 1)], f16)
            normfe_t = cpool.tile([P, max(NFULL, 1)], f16)
            normfo_t = cpool.tile([P, max(NFULL, 1)], f16)
            selfx_t = cpool.tile([P, SHARDP], f16)
            ident_t = cpool.tile([P, P], f16)
            iota_t = cpool.tile([P, P], f16)
            wgcnT_t = cpool.tile([IN_C, HID], f16)
            wihT_t = cpool.tile([P, 3 * HID], f16)
            whhT_t = cpool.tile([P, 3 * HID], f16)
            br_t = cpool.tile([P, 1], f32)
            bz_t = cpool.tile([P, 1], f32)
            bihn_t = cpool.tile([P, 1], f32)
            bhhn_t = cpool.tile([P, 1], f32)
            hmemT_t = cpool.tile([HID, SHARDP], f16)

            # group-0 idx slice first so gather 0 launches ASAP
            i0, i1 = gt0[0] * 8, (gt0[0] + gnt[0]) * 8
            nc.sync.dma_start(out=idx_t[:, i0:i1], in_=idx_d[:, i0:i1])

            for t, d in [(ident_t, ident_d), (iota_t, iota_d), (dstn_t, dstn_d),
                         (norme_t, norme_d), (normo_t, normo_d), (dstf_t, dstf_d),
                         (normfe_t, normfe_d), (normfo_t, normfo_d)]:
                nc.sync.dma_start(out=t[:], in_=d[:])
            loads = [(wgcnT_t, wgcnT_d), (wihT_t, wihT_d),
                     (br_t, br_d), (bz_t, bz_d), (bihn_t, bihn_d), (bhhn_t, bhhn_d)]
            if not hzero:
                loads += [(whhT_t, whhT_d), (hmemT_t, hmemT_d)]

            agg_tiles = []
            for c in range(NCH):
                a_t = apool.tile([P, min(512, SHARDP - c * 512)], f16, name=f"agg{c}")
                agg_tiles.append(a_t)

            iota3n = iota_t[:, 0:W].rearrange("p (a j) -> p a j", a=1)
            iota3f = iota_t[:].rearrange("p (a j) -> p a j", a=1)

            def do_group(gi):
                g = gblocks[gi]
                t0, nt, f0, nf = gt0[gi], gnt[gi], gf0[gi], gnf[gi]
                span = nt * P

                if gi > 0:
                    i0, i1 = t0 * 8, (t0 + nt) * 8
                    nc.sync.dma_start(out=idx_t[:, i0:i1], in_=idx_d[:, i0:i1])
                g_t = gpool.tile([P, Tmax, 2 * IN_C], f16, tag="g")
                nc.gpsimd.dma_gather(
                    g_t[:, 0:nt, :], x16_d[:],
                    idx_t[:, t0 * 8:(t0 + nt) * 8],
                    span, span, 2 * IN_C, single_packet=False)
                # self rows for this group's blocks (contiguous slice)
                c0, c1 = g[0] * P, (g[-1] + 1) * P
                nc.sync.dma_start(out=selfx_t[:, c0:c1], in_=selfx_d[:, c0:c1])

                if 'sbuild' in skip:
                    return
                se_t = spool.tile([P, Tmax, W], f16, tag="se")
                so_t = spool.tile([P, Tmax, W], f16, tag="so")
                nc.vector.tensor_tensor(
                    out=se_t[:, 0:nt, :],
                    in0=iota3n.to_broadcast([P, nt, W]),
                    in1=dstn_t[:, t0:t0 + nt].to_broadcast([P, nt, W]),
                    op=mybir.AluOpType.is_equal)
                nc.vector.tensor_tensor(
                    out=so_t[:, 0:nt, :], in0=se_t[:, 0:nt, :],
                    in1=normo_t[:, t0:t0 + nt].to_broadcast([P, nt, W]),
                    op=mybir.AluOpType.mult)
                nc.vector.tensor_tensor(
                    out=se_t[:, 0:nt, :], in0=se_t[:, 0:nt, :],
                    in1=norme_t[:, t0:t0 + nt].to_broadcast([P, nt, W]),
                    op=mybir.AluOpType.mult)
                sfe_t = sfo_t = None
                if nf:
                    sfe_t = sfpool.tile([P, NFmax, P], f16, tag="sfe")
                    sfo_t = sfpool.tile([P, NFmax, P], f16, tag="sfo")
                    nc.vector.tensor_tensor(
                        out=sfe_t[:, 0:nf, :],
                        in0=iota3f.to_broadcast([P, nf, P]),
                        in1=dstf_t[:, f0:f0 + nf].to_broadcast([P, nf, P]),
                        op=mybir.AluOpType.is_equal)
                    nc.vector.tensor_tensor(
                        out=sfo_t[:, 0:nf, :], in0=sfe_t[:, 0:nf, :],
                        in1=normfo_t[:, f0:f0 + nf].to_broadcast([P, nf, P]),
                        op=mybir.AluOpType.mult)
                    nc.vector.tensor_tensor(
                        out=sfe_t[:, 0:nf, :], in0=sfe_t[:, 0:nf, :],
                        in1=normfe_t[:, f0:f0 + nf].to_broadcast([P, nf, P]),
                        op=mybir.AluOpType.mult)

                if 'mm' in skip:
                    return
                for b in g:
                    nmm = 1 + 2 * int(Tb[b])
                    psum_t = ppool.tile([P, P], f32, space="PSUM", tag="ps")
                    nc.tensor.matmul(
                        out=psum_t[:], lhsT=selfx_t[:, b * P:(b + 1) * P],
                        rhs=ident_t[:], start=True, stop=(nmm == 1),
                        skip_group_check=True)
                    k = 1
                    bt0 = int(slot_off[b]) // P
                    for t in range(int(Tb[b])):
                        ti = bt0 + t
                        rel = ti - t0
                        if tile_full[ti]:
                            rf = int(full_idx[ti]) - f0
                            rhs_e, rhs_o = sfe_t[:, rf, :], sfo_t[:, rf, :]
                            oe = oo = psum_t[:]
                        else:
                            rhs_e, rhs_o = se_t[:, rel, :], so_t[:, rel, :]
                            ws = int(w0[ti])
                            oe = oo = psum_t[:, ws:ws + W]
                        nc.tensor.matmul(out=oe, lhsT=g_t[:, rel, 0:IN_C],
                                         rhs=rhs_e, start=False, stop=False,
                                         skip_group_check=True)
                        k += 1
                        nc.tensor.matmul(out=oo, lhsT=g_t[:, rel, IN_C:2 * IN_C],
                                         rhs=rhs_o, start=False,
                                         stop=(k == nmm), skip_group_check=True)
                        k += 1
                    nc.scalar.copy(
                        out=agg_tiles[b // 4][:, (b % 4) * P:(b % 4 + 1) * P],
                        in_=psum_t[:])

            AF = mybir.ActivationFunctionType

            def do_chunk(c):
                wc = min(512, SHARDP - c * 512)
                c0 = c * 512

                gcn_ps = pp2g.tile([HID, 512], f32, space="PSUM", tag="gcn")
                nc.tensor.matmul(out=gcn_ps[:, 0:wc], lhsT=wgcnT_t[:],
                                 rhs=agg_tiles[c][:, 0:wc], start=True, stop=True)
                gcn_sb = sb2.tile([HID, 512], f16, tag="gcnsb")
                nc.scalar.copy(out=gcn_sb[:, 0:wc], in_=gcn_ps[:, 0:wc])

                def gate_mm(tag, wslice):
                    ps = pp2.tile([HID, 512], f32, space="PSUM", tag=tag)
                    nc.tensor.matmul(out=ps[:, 0:wc], lhsT=wihT_t[0:HID, wslice],
                                     rhs=gcn_sb[:, 0:wc], start=True, stop=HZERO)
                    if not HZERO:
                        nc.tensor.matmul(out=ps[:, 0:wc], lhsT=whhT_t[0:HID, wslice],
                                         rhs=hmemT_t[0:HID, c0:c0 + wc],
                                         start=False, stop=True)
                    return ps

                r_ps = gate_mm("r", slice(0, HID))
                z_ps = gate_mm("z", slice(HID, 2 * HID))
                n_ps = pp2.tile([HID, 512], f32, space="PSUM", tag="n")
                nc.tensor.matmul(out=n_ps[:, 0:wc], lhsT=wihT_t[0:HID, 2 * HID:3 * HID],
                                 rhs=gcn_sb[:, 0:wc], start=True, stop=True)

                r_sb = sb2.tile([HID, 512], f16, tag="r_sb")
                nc.scalar.activation(out=r_sb[:, 0:wc], in_=r_ps[:, 0:wc],
                                     func=AF.Sigmoid, bias=br_t[0:HID, :])
                z_sb = sb2.tile([HID, 512], f16, tag="z_sb")
                nc.scalar.activation(out=z_sb[:, 0:wc], in_=z_ps[:, 0:wc],
                                     func=AF.Sigmoid, bias=bz_t[0:HID, :])

                rhn = sb2.tile([HID, 512], f16, tag="rhn")
                if HZERO:
                    nc.vector.tensor_scalar(out=rhn[:, 0:wc], in0=r_sb[:, 0:wc],
                                            scalar1=bhhn_t[0:HID, :], scalar2=None,
                                            op0=mybir.AluOpType.mult)
                else:
                    hn_ps = pp2.tile([HID, 512], f32, space="PSUM", tag="hn")
                    nc.tensor.matmul(out=hn_ps[:, 0:wc],
                                     lhsT=whhT_t[0:HID, 2 * HID:3 * HID],
                                     rhs=hmemT_t[0:HID, c0:c0 + wc],
                                     start=True, stop=True)
                    hn_sb = sb2.tile([HID, 512], f16, tag="hn_sb")
                    nc.scalar.activation(out=hn_sb[:, 0:wc], in_=hn_ps[:, 0:wc],
                                         func=AF.Identity, bias=bhhn_t[0:HID, :])
                    nc.vector.tensor_mul(out=rhn[:, 0:wc], in0=r_sb[:, 0:wc],
                                         in1=hn_sb[:, 0:wc])

                pre = sb2.tile([HID, 512], f16, tag="pre")
                nc.vector.tensor_add(out=pre[:, 0:wc], in0=rhn[:, 0:wc],
                                     in1=n_ps[:, 0:wc])
                nact = sb2.tile([HID, 512], f16, tag="nact")
                nc.scalar.activation(out=nact[:, 0:wc], in_=pre[:, 0:wc],
                                     func=AF.Tanh, bias=bihn_t[0:HID, :])

                h_sb = sb2.tile([HID, 512], f16, tag="h_sb")
                if HZERO:
                    zn = sb2.tile([HID, 512], f16, tag="zn")
                    nc.vector.tensor_mul(out=zn[:, 0:wc], in0=z_sb[:, 0:wc],
                                         in1=nact[:, 0:wc])
                    nc.vector.tensor_sub(out=h_sb[:, 0:wc], in0=nact[:, 0:wc],
                                         in1=zn[:, 0:wc])
                else:
                    d_sb = sb2.tile([HID, 512], f16, tag="d_sb")
                    nc.vector.tensor_sub(out=d_sb[:, 0:wc],
                                         in0=hmemT_t[0:HID, c0:c0 + wc],
                                         in1=nact[:, 0:wc])
                    e_sb = sb2.tile([HID, 512], f16, tag="e_sb")
                    nc.vector.tensor_mul(out=e_sb[:, 0:wc], in0=z_sb[:, 0:wc],
                                         in1=d_sb[:, 0:wc])
                    nc.vector.tensor_add(out=h_sb[:, 0:wc], in0=nact[:, 0:wc],
                                         in1=e_sb[:, 0:wc])
                nc.sync.dma_start(out=out_d[:, c0:c0 + wc], in_=h_sb[:, 0:wc])

            do_group(0)
            for t, d in loads:
                nc.sync.dma_start(out=t[:], in_=d[:])
            blocks_done = GS[0]
            pair_next = 0
            for gi in range(1, len(GS)):
                do_group(gi)
                blocks_done += GS[gi]
                if 'phase2' in skip:
                    continue
                while pair_next < NCH and blocks_done >= min(4 * pair_next + 4 + PAIR_LAG, BLOCKS):
                    do_chunk(pair_next)
                    pair_next += 1
            if 'phase2' not in skip:
                while pair_next < NCH:
                    do_chunk(pair_next)
                    pair_next += 1

    return nc


def kernel(x, edge_index, edge_weight, W_gcn, b_gcn, W_ih, W_hh, b_ih, b_hh, h_mem):
    global last_nc
    import concourse.mybir as mybir
    from concourse.bass_utils import run_bass_kernel_spmd

    x = np.asarray(x, dtype=np.float32)
    h_mem = np.asarray(h_mem, dtype=np.float32)
    W_gcn = np.asarray(W_gcn, dtype=np.float32)
    W_ih = np.asarray(W_ih, dtype=np.float32)
    W_hh = np.asarray(W_hh, dtype=np.float32)
    b_gcn = np.asarray(b_gcn, dtype=np.float32)
    b_ih = np.asarray(b_ih, dtype=np.float32)
    b_hh = np.asarray(b_hh, dtype=np.float32)

    src = np.asarray(edge_index[0], dtype=np.int64)
    dst = np.asarray(edge_index[1], dtype=np.int64)
    w = np.asarray(edge_weight, dtype=np.float32)
    deg = (np.bincount(dst, weights=w.astype(np.float64), minlength=N)
           .astype(np.float32) + np.float32(1.0))
    dinv2 = (np.float32(1.0) / deg).astype(np.float32)

    pl, idx_w, dstnT, normeT, normoT, dstF, normFe, normFo = _host_prep(
        x, edge_index, edge_weight)

    hzero = not np.any(h_mem)
    nc = _build_program(pl, hzero=hzero)
    last_nc = nc

    mybir.codegen_inst_isa_subclasses(nc)
    _split_sync_waits(nc)

    x16 = x.astype(np.float16)
    x16_pairs = np.ascontiguousarray(x16.reshape(N // 2, 2 * IN_C))

    # selfx[c][p, b*128+f] = dinv2[v] * x[v, f], v = node_at[c, b*128+p]
    selfx = np.zeros((CORES, P, SHARDP), np.float16)
    na = pl.node_at                              # [CORES, SHARDP]
    for c in range(CORES):
        nav = na[c].reshape(BLOCKS, P)
        for b in range(BLOCKS):
            vs = nav[b]
            ok = vs >= 0
            rows = np.zeros((P, IN_C), np.float32)
            rows[ok] = x[vs[ok]] * dinv2[vs[ok]][:, None]
            selfx[c, :, b * P:(b + 1) * P] = rows.astype(np.float16)

    ident = np.eye(P, dtype=np.float16)
    iota_np = np.broadcast_to(np.arange(P, dtype=np.float16), (P, P)).copy()

    b_ihp = (b_ih + W_ih @ b_gcn).astype(np.float32)
    br = np.tile((b_ihp[0:HID] + b_hh[0:HID]).astype(np.float32), 2).reshape(P, 1)
    bz = np.tile((b_ihp[HID:2 * HID] + b_hh[HID:2 * HID]).astype(np.float32), 2).reshape(P, 1)
    bihn = np.tile(b_ihp[2 * HID:3 * HID].astype(np.float32), 2).reshape(P, 1)
    bhhn = np.tile(b_hh[2 * HID:3 * HID].astype(np.float32), 2).reshape(P, 1)

    wgcnT = np.ascontiguousarray(W_gcn.T).astype(np.float16)
    wihT = np.ascontiguousarray(np.vstack([W_ih.T, W_ih.T])).astype(np.float16)
    whhT = np.ascontiguousarray(np.vstack([W_hh.T, W_hh.T])).astype(np.float16)

    hmemT = np.zeros((CORES, HID, SHARDP), np.float16)
    if np.any(h_mem):
        for c in range(CORES):
            ok = na[c] >= 0
            hmemT[c][:, ok] = h_mem[na[c][ok]].T.astype(np.float16)

    in_maps = []
    for c in range(CORES):
        in_maps.append({
            "x16": x16_pairs, "idx": idx_w[c], "dstn": dstnT[c],
            "norme": normeT[c], "normo": normoT[c],
            "dstf": dstF[c], "normfe": normFe[c], "normfo": normFo[c],
            "selfx": selfx[c], "ident": ident, "iota": iota_np,
            "wgcnT": wgcnT, "wihT": wihT, "whhT": whhT,
            "br": br, "bz": bz, "bihn": bihn, "bhhn": bhhn, "hmemT": hmemT[c],
        })

    res = run_bass_kernel_spmd(nc, in_maps, core_ids=list(range(CORES)))
    out = np.empty((N, HID), np.float32)
    for c in range(CORES):
        o = res.results[c]["outT"].astype(np.float32)    # [HID, SHARDP]
        ok = na[c] >= 0
        out[na[c][ok]] = o[:, ok].T
    return out


# revision 4
# speedup vs baseline: 1.0045x; 1.0045x over previous
"""TGCN (GCNConv + GRUCell) Bass kernel for 8 TRN2 NeuronCores — v2.

vs baseline: fp16 pair-gather (512B descriptors, idx = src>>1 so no src-half
split), LPT node rebalancing to equalize per-(core,block) edge counts, self
loops via contiguous DMA of host-prescaled dinv^2*x rows (identity-matrix
matmul, start=True per block), fp16 matmuls/S-tiles/GRU throughout.
"""
import sys

sys.path.insert(0, '/opt/trn_rl_repo')

import heapq
import numpy as np

N = 50000
E_IN = 800000
IN_C = 128
HID = 64
CORES = 8
P = 128
BLOCKS = 49                 # blocks of 128 dst per core
SHARDP = BLOCKS * P         # 6272 padded shard width (bins may exceed 6250)
W = 16                      # narrow one-hot window width
GS = [2, 3] + [5] * 8 + [2, 1, 1]  # blocks per gather group (sum = 49)
PAIR_LAG = 4                # extra blocks required beyond a pair's own range
NCH = 13                    # 512-wide phase-2 chunks (12x512 + 128)
NPAIR = 7

last_nc = None

_splitwait_uid = [0]


def _split_sync_waits(nc, limit=1):
    """Walrus rejects >1 semaphore wait per instruction; move extras onto
    same-engine NoOp carriers inserted immediately before."""
    import concourse.mybir as mybir
    n_fixed = 0
    for f in nc.m.functions:
        for bb in f.blocks:
            insts = bb.instructions
            i = 0
            while i < len(insts):
                inst = insts[i]
                si = inst.sync_info
                if si is not None and si.on_wait is not None and len(si.on_wait) > limit:
                    waits = list(si.on_wait)
                    pre = []
                    while len(waits) > limit:
                        chunk, waits = waits[:limit], waits[limit:]
                        _splitwait_uid[0] += 1
                        pre.append(mybir.InstNoOp(
                            name=f"bass_splitwait_{_splitwait_uid[0]}",
                            engine=inst.engine,
                            sync_info=mybir.SyncInfo(on_wait=chunk, on_update=[]),
                        ))
                    si.on_wait = waits
                    for j, nd in enumerate(pre):
                        insts.insert(i + j, nd)
                    i += len(pre)
                    n_fixed += 1
                i += 1
    return n_fixed


class Plan:
    pass


def _balance_nodes(edeg):
    """LPT: assign N nodes into CORES*BLOCKS bins of exactly 128 nodes,
    balancing per-bin edge sums. Returns bin id per node and slot order."""
    NB = CORES * BLOCKS
    order = np.argsort(-edeg, kind='stable')
    heap = [(0, b) for b in range(NB)]
    heapq.heapify(heap)
    cnt = np.zeros(NB, np.int64)
    assign = np.empty(N, np.int64)
    dib = np.empty(N, np.int64)
    for v in order:
        while True:
            s, b = heapq.heappop(heap)
            if cnt[b] < P:
                break
        assign[v] = b
        dib[v] = cnt[b]
        cnt[b] += 1
        if cnt[b] < P:
            heapq.heappush(heap, (s + int(edeg[v]), b))
    return assign, dib


def _host_prep(x, edge_index, edge_weight):
    src = np.asarray(edge_index[0], dtype=np.int64)
    dst = np.asarray(edge_index[1], dtype=np.int64)
    w = np.asarray(edge_weight, dtype=np.float32)

    deg = (np.bincount(dst, weights=w.astype(np.float64), minlength=N)
           .astype(np.float32) + np.float32(1.0))
    dinv = (np.float32(1.0) / np.sqrt(deg)).astype(np.float32)
    norm = (dinv[src] * w * dinv[dst]).astype(np.float32)

    edeg = np.bincount(dst, minlength=N)
    abin, adib = _balance_nodes(edeg)
    acore = abin // BLOCKS
    ablk = abin % BLOCKS
    apos = ablk * P + adib                        # position within core shard
    # node_at[c, pos] = node (or -1)
    node_at = np.full((CORES, SHARDP), -1, np.int64)
    node_at[acore, apos] = np.arange(N)

    ec = acore[dst]
    eb = ablk[dst]
    edib = adib[dst]
    parity = (src & 1).astype(np.int64)
    idxp = (src >> 1).astype(np.int16)

    key = ec * BLOCKS + eb
    order = np.lexsort((edib, key))
    s_key = key[order]
    s_idx = idxp[order]
    s_dib = edib[order]
    s_norm = norm[order]
    s_par = parity[order]

    counts = np.bincount(s_key, minlength=CORES * BLOCKS).reshape(CORES, BLOCKS)
    Tb = (counts.max(axis=0) + P - 1) // P        # tiles per block
    PB = Tb * P
    slot_off = np.zeros(BLOCKS, np.int64)
    slot_off[1:] = np.cumsum(PB)[:-1]
    TOT = int(PB.sum())
    Ttot = TOT // P

    grp_start = np.zeros(CORES * BLOCKS + 1, np.int64)
    np.cumsum(np.bincount(s_key, minlength=CORES * BLOCKS), out=grp_start[1:])
    rank = np.arange(len(s_key)) - grp_start[s_key]
    b_of = s_key % BLOCKS
    c_of = s_key // BLOCKS
    slot = slot_off[b_of] + rank

    idx_flat = np.zeros((CORES, TOT), np.int16)
    dib_flat = np.full((CORES, TOT), -1, np.int64)
    norme = np.zeros((CORES, TOT), np.float32)
    normo = np.zeros((CORES, TOT), np.float32)
    idx_flat[c_of, slot] = s_idx
    dib_flat[c_of, slot] = s_dib
    norme[c_of, slot] = np.where(s_par == 0, s_norm, 0.0)
    normo[c_of, slot] = np.where(s_par == 1, s_norm, 0.0)

    # per-tile dst range union over cores
    dib_t = dib_flat.reshape(CORES, Ttot, P)
    val_t = dib_t >= 0
    tmin = np.where(val_t, dib_t, 10**6).min(axis=(0, 2))
    tmax = np.where(val_t, dib_t, -1).max(axis=(0, 2))
    empty = tmax < 0
    tmin[empty] = 0
    tmax[empty] = 0
    tile_full = (tmax - tmin) >= W
    w0 = np.minimum(tmin, P - W)
    w0[tile_full] = 0
    full_idx = np.cumsum(tile_full) - 1
    NFULL = int(tile_full.sum())

    w0_slot = w0[np.arange(TOT) // P]
    dstn = (dib_flat - w0_slot[None, :]).astype(np.float32)
    dstn[dib_flat < 0] = -1.0
    dibF = dib_flat.astype(np.float32)
    dibF[dib_flat < 0] = -1.0

    idx_w = np.ascontiguousarray(
        np.tile(idx_flat.reshape(CORES, -1, 16).transpose(0, 2, 1), (1, 8, 1)))
    dstnT = np.ascontiguousarray(
        dstn.reshape(CORES, Ttot, P).transpose(0, 2, 1)).astype(np.float16)
    normeT = np.ascontiguousarray(
        norme.reshape(CORES, Ttot, P).transpose(0, 2, 1)).astype(np.float16)
    normoT = np.ascontiguousarray(
        normo.reshape(CORES, Ttot, P).transpose(0, 2, 1)).astype(np.float16)
    ft = np.flatnonzero(tile_full)
    dstF = np.ascontiguousarray(
        dibF.reshape(CORES, Ttot, P).transpose(0, 2, 1)[:, :, ft]).astype(np.float16)
    normFe = np.ascontiguousarray(
        norme.reshape(CORES, Ttot, P).transpose(0, 2, 1)[:, :, ft]).astype(np.float16)
    normFo = np.ascontiguousarray(
        normo.reshape(CORES, Ttot, P).transpose(0, 2, 1)[:, :, ft]).astype(np.float16)
    if NFULL == 0:
        z = np.full((CORES, P, 1), -1.0, np.float16)
        dstF, normFe, normFo = z, np.zeros_like(z), np.zeros_like(z)

    pl = Plan()
    pl.Tb, pl.slot_off, pl.Ttot, pl.TOT = Tb, slot_off, Ttot, TOT
    pl.tile_full, pl.w0, pl.full_idx, pl.NFULL = tile_full, w0, full_idx, NFULL
    pl.node_at = node_at
    return pl, idx_w, dstnT, normeT, normoT, dstF, normFe, normFo


def _build_program(pl, hzero=True, skip=()):
    import concourse.bass as bass
    import concourse.tile as tile
    import concourse.mybir as mybir
    from concourse import library_config

    Tb, slot_off, Ttot = pl.Tb, pl.slot_off, pl.Ttot
    tile_full, w0, full_idx, NFULL = pl.tile_full, pl.w0, pl.full_idx, pl.NFULL
    full_pos = np.flatnonzero(tile_full)
    HZERO = bool(hzero)

    f16 = mybir.dt.float16
    f32 = mybir.dt.float32
    PH = NPAIR * 512                      # 3584 packed phase-2 columns

    # group structure: block ranges + tile ranges + full-tile ranges
    gblocks, b0 = [], 0
    for gs in GS:
        gblocks.append(list(range(b0, b0 + gs)))
        b0 += gs
    gt0 = [int(slot_off[g[0]]) // P for g in gblocks]          # first tile
    gnt = [int(sum(Tb[b] for b in g)) for g in gblocks]        # tiles in group
    gf0 = [int(np.searchsorted(full_pos, t)) for t in gt0]
    gnf = [int(tile_full[gt0[i]:gt0[i] + gnt[i]].sum()) for i in range(len(GS))]
    Tmax = max(gnt)
    NFmax = max(max(gnf), 1)

    nc = bass.Bass("TRN2", target_bir_lowering=False, debug=False,
                   num_devices=CORES)

    x16_d = nc.dram_tensor("x16", [N // 2, 2 * IN_C], f16, kind="ExternalInput")
    idx_d = nc.dram_tensor("idx", [P, pl.TOT // 16], mybir.dt.int16, kind="ExternalInput")
    dstn_d = nc.dram_tensor("dstn", [P, Ttot], f16, kind="ExternalInput")
    norme_d = nc.dram_tensor("norme", [P, Ttot], f16, kind="ExternalInput")
    normo_d = nc.dram_tensor("normo", [P, Ttot], f16, kind="ExternalInput")
    dstf_d = nc.dram_tensor("dstf", [P, max(NFULL, 1)], f16, kind="ExternalInput")
    normfe_d = nc.dram_tensor("normfe", [P, max(NFULL, 1)], f16, kind="ExternalInput")
    normfo_d = nc.dram_tensor("normfo", [P, max(NFULL, 1)], f16, kind="ExternalInput")
    selfx_d = nc.dram_tensor("selfx", [P, SHARDP], f16, kind="ExternalInput")
    ident_d = nc.dram_tensor("ident", [P, P], f16, kind="ExternalInput")
    iota_d = nc.dram_tensor("iota", [P, P], f16, kind="ExternalInput")
    wgcnT_d = nc.dram_tensor("wgcnT", [IN_C, HID], f16, kind="ExternalInput")
    wihT_d = nc.dram_tensor("wihT", [P, 3 * HID], f16, kind="ExternalInput")
    whhT_d = nc.dram_tensor("whhT", [P, 3 * HID], f16, kind="ExternalInput")
    br_d = nc.dram_tensor("br", [P, 1], f32, kind="ExternalInput")
    bz_d = nc.dram_tensor("bz", [P, 1], f32, kind="ExternalInput")
    bihn_d = nc.dram_tensor("bihn", [P, 1], f32, kind="ExternalInput")
    bhhn_d = nc.dram_tensor("bhhn", [P, 1], f32, kind="ExternalInput")
    hmemT_d = nc.dram_tensor("hmemT", [HID, SHARDP], f16, kind="ExternalInput")
    out_d = nc.dram_tensor("outT", [HID, SHARDP], f16, kind="ExternalOutput")

    with tile.TileContext(nc, trace_sim=False) as tc:
        nc.gpsimd.load_library(library_config.mlp)
        with (
            tc.tile_pool(name="const", bufs=1) as cpool,
            tc.tile_pool(name="agg", bufs=1) as apool,
            tc.tile_pool(name="g", bufs=2) as gpool,
            tc.tile_pool(name="s", bufs=2) as spool,
            tc.tile_pool(name="sf", bufs=2) as sfpool,
            tc.tile_pool(name="ps1", bufs=3, space="PSUM") as ppool,
            tc.tile_pool(name="p2", bufs=2) as sb2,
            tc.tile_pool(name="ps2g", bufs=2, space="PSUM") as pp2g,
            tc.tile_pool(name="ps2", bufs=1, space="PSUM") as pp2,
        ):
            idx_t = cpool.tile([P, pl.TOT // 16], mybir.dt.int16)
            dstn_t = cpool.tile([P, Ttot], f16)
            norme_t = cpool.tile([P, Ttot], f16)
            normo_t = cpool.tile([P, Ttot], f16)
            dstf_t = cpool.tile([P, max(NFULL, 1)], f16)
            normfe_t = cpool.tile([P, max(NFULL, 1)], f16)
            normfo_t = cpool.tile([P, max(NFULL, 1)], f16)
            selfx_t = cpool.tile([P, SHARDP], f16)
            ident_t = cpool.tile([P, P], f16)
            iota_t = cpool.tile([P, P], f16)
            wgcnT_t = cpool.tile([IN_C, HID], f16)
            wihT_t = cpool.tile([P, 3 * HID], f16)
            whhT_t = cpool.tile([P, 3 * HID], f16)
            br_t = cpool.tile([P, 1], f32)
            bz_t = cpool.tile([P, 1], f32)
            bihn_t = cpool.tile([P, 1], f32)
            bhhn_t = cpool.tile([P, 1], f32)
            hmemT_t = cpool.tile([HID, SHARDP], f16)

            # group-0 idx slice first so gather 0 launches ASAP
            i0, i1 = gt0[0] * 8, (gt0[0] + gnt[0]) * 8
            nc.sync.dma_start(out=idx_t[:, i0:i1], in_=idx_d[:, i0:i1])

            for t, d in [(ident_t, ident_d), (iota_t, iota_d), (dstn_t, dstn_d),
                         (norme_t, norme_d), (normo_t, normo_d), (dstf_t, dstf_d),
                         (normfe_t, normfe_d), (normfo_t, normfo_d)]:
                nc.sync.dma_start(out=t[:], in_=d[:])
            loads = [(wgcnT_t, wgcnT_d), (wihT_t, wihT_d),
                     (br_t, br_d), (bz_t, bz_d), (bihn_t, bihn_d), (bhhn_t, bhhn_d)]
            if not hzero:
                loads += [(whhT_t, whhT_d), (hmemT_t, hmemT_d)]

            agg_tiles = []
            for c in range(NCH):
                a_t = apool.tile([P, min(512, SHARDP - c * 512)], f16, name=f"agg{c}")
                agg_tiles.append(a_t)

            iota3n = iota_t[:, 0:W].rearrange("p (a j) -> p a j", a=1)
            iota3f = iota_t[:].rearrange("p (a j) -> p a j", a=1)

            def do_group(gi):
                g = gblocks[gi]
                t0, nt, f0, nf = gt0[gi], gnt[gi], gf0[gi], gnf[gi]
                span = nt * P

                if gi > 0:
                    i0, i1 = t0 * 8, (t0 + nt) * 8
                    nc.sync.dma_start(out=idx_t[:, i0:i1], in_=idx_d[:, i0:i1])
                g_t = gpool.tile([P, Tmax, 2 * IN_C], f16, tag="g")
                nc.gpsimd.dma_gather(
                    g_t[:, 0:nt, :], x16_d[:],
                    idx_t[:, t0 * 8:(t0 + nt) * 8],
                    span, span, 2 * IN_C, single_packet=False)
                # self rows for this group's blocks (contiguous slice)
                c0, c1 = g[0] * P, (g[-1] + 1) * P
                nc.sync.dma_start(out=selfx_t[:, c0:c1], in_=selfx_d[:, c0:c1])

                if 'sbuild' in skip:
                    return
                se_t = spool.tile([P, Tmax, W], f16, tag="se")
                so_t = spool.tile([P, Tmax, W], f16, tag="so")
                nc.vector.tensor_tensor(
                    out=se_t[:, 0:nt, :],
                    in0=iota3n.to_broadcast([P, nt, W]),
                    in1=dstn_t[:, t0:t0 + nt].to_broadcast([P, nt, W]),
                    op=mybir.AluOpType.is_equal)
                nc.vector.tensor_tensor(
                    out=so_t[:, 0:nt, :], in0=se_t[:, 0:nt, :],
                    in1=normo_t[:, t0:t0 + nt].to_broadcast([P, nt, W]),
                    op=mybir.AluOpType.mult)
                nc.vector.tensor_tensor(
                    out=se_t[:, 0:nt, :], in0=se_t[:, 0:nt, :],
                    in1=norme_t[:, t0:t0 + nt].to_broadcast([P, nt, W]),
                    op=mybir.AluOpType.mult)
                sfe_t = sfo_t = None
                if nf:
                    sfe_t = sfpool.tile([P, NFmax, P], f16, tag="sfe")
                    sfo_t = sfpool.tile([P, NFmax, P], f16, tag="sfo")
                    nc.vector.tensor_tensor(
                        out=sfe_t[:, 0:nf, :],
                        in0=iota3f.to_broadcast([P, nf, P]),
                        in1=dstf_t[:, f0:f0 + nf].to_broadcast([P, nf, P]),
                        op=mybir.AluOpType.is_equal)
                    nc.vector.tensor_tensor(
                        out=sfo_t[:, 0:nf, :], in0=sfe_t[:, 0:nf, :],
                        in1=normfo_t[:, f0:f0 + nf].to_broadcast([P, nf, P]),
                        op=mybir.AluOpType.mult)
                    nc.vector.tensor_tensor(
                        out=sfe_t[:, 0:nf, :], in0=sfe_t[:, 0:nf, :],
                        in1=normfe_t[:, f0:f0 + nf].to_broadcast([P, nf, P]),
                        op=mybir.AluOpType.mult)

                if 'mm' in skip:
                    return
                for b in g:
                    nmm = 1 + 2 * int(Tb[b])
                    psum_t = ppool.tile([P, P], f32, space="PSUM", tag="ps")
                    nc.tensor.matmul(
                        out=psum_t[:], lhsT=selfx_t[:, b * P:(b + 1) * P],
                        rhs=ident_t[:], start=True, stop=(nmm == 1),
                        skip_group_check=True)
                    k = 1
                    bt0 = int(slot_off[b]) // P
                    for t in range(int(Tb[b])):
                        ti = bt0 + t
                        rel = ti - t0
                        if tile_full[ti]:
                            rf = int(full_idx[ti]) - f0
                            rhs_e, rhs_o = sfe_t[:, rf, :], sfo_t[:, rf, :]
                            oe = oo = psum_t[:]
                        else:
                            rhs_e, rhs_o = se_t[:, rel, :], so_t[:, rel, :]
                            ws = int(w0[ti])
                            oe = oo = psum_t[:, ws:ws + W]
                        nc.tensor.matmul(out=oe, lhsT=g_t[:, rel, 0:IN_C],
                                         rhs=rhs_e, start=False, stop=False,
                                         skip_group_check=True)
                        k += 1
                        nc.tensor.matmul(out=oo, lhsT=g_t[:, rel, IN_C:2 * IN_C],
                                         rhs=rhs_o, start=False,
                                         stop=(k == nmm), skip_group_check=True)
                        k += 1
                    nc.scalar.copy(
                        out=agg_tiles[b // 4][:, (b % 4) * P:(b % 4 + 1) * P],
                        in_=psum_t[:])

            AF = mybir.ActivationFunctionType

            def do_chunk(c):
                wc = min(512, SHARDP - c * 512)
                c0 = c * 512

                gcn_ps = pp2g.tile([HID, 512], f32, space="PSUM", tag="gcn")
                nc.tensor.matmul(out=gcn_ps[:, 0:wc], lhsT=wgcnT_t[:],
                                 rhs=agg_tiles[c][:, 0:wc], start=True, stop=True)
                gcn_sb = sb2.tile([HID, 512], f16, tag="gcnsb")
                nc.scalar.copy(out=gcn_sb[:, 0:wc], in_=gcn_ps[:, 0:wc])

                def gate_mm(tag, wslice):
                    ps = pp2.tile([HID, 512], f32, space="PSUM", tag=tag)
                    nc.tensor.matmul(out=ps[:, 0:wc], lhsT=wihT_t[0:HID, wslice],
                                     rhs=gcn_sb[:, 0:wc], start=True, stop=HZERO)
                    if not HZERO:
                        nc.tensor.matmul(out=ps[:, 0:wc], lhsT=whhT_t[0:HID, wslice],
                                         rhs=hmemT_t[0:HID, c0:c0 + wc],
                                         start=False, stop=True)
                    return ps

                r_ps = gate_mm("r", slice(0, HID))
                z_ps = gate_mm("z", slice(HID, 2 * HID))
                n_ps = pp2.tile([HID, 512], f32, space="PSUM", tag="n")
                nc.tensor.matmul(out=n_ps[:, 0:wc], lhsT=wihT_t[0:HID, 2 * HID:3 * HID],
                                 rhs=gcn_sb[:, 0:wc], start=True, stop=True)

                r_sb = sb2.tile([HID, 512], f16, tag="r_sb")
                nc.scalar.activation(out=r_sb[:, 0:wc], in_=r_ps[:, 0:wc],
                                     func=AF.Sigmoid, bias=br_t[0:HID, :])
                z_sb = sb2.tile([HID, 512], f16, tag="z_sb")
                nc.scalar.activation(out=z_sb[:, 0:wc], in_=z_ps[:, 0:wc],
                                     func=AF.Sigmoid, bias=bz_t[0:HID, :])

                rhn = sb2.tile([HID, 512], f16, tag="rhn")
                if HZERO:
                    nc.vector.tensor_scalar(out=rhn[:, 0:wc], in0=r_sb[:, 0:wc],
                                            scalar1=bhhn_t[0:HID, :], scalar2=None,
                                            op0=mybir.AluOpType.mult)
                else:
                    hn_ps = pp2.tile([HID, 512], f32, space="PSUM", tag="hn")
                    nc.tensor.matmul(out=hn_ps[:, 0:wc],
                                     lhsT=whhT_t[0:HID, 2 * HID:3 * HID],
                                     rhs=hmemT_t[0:HID, c0:c0 + wc],
                                     start=True, stop=True)
                    hn_sb = sb2.tile([HID, 512], f16, tag="hn_sb")
                    nc.scalar.activation(out=hn_sb[:, 0:wc], in_=hn_ps[:, 0:wc],
                                         func=AF.Identity, bias=bhhn_t[0:HID, :])
                    nc.vector.tensor_mul(out=rhn[:, 0:wc], in0=r_sb[:, 0:wc],
                                         in1=hn_sb[:, 0:wc])

                pre = sb2.tile([HID, 512], f16, tag="pre")
                nc.vector.tensor_add(out=pre[:, 0:wc], in0=rhn[:, 0:wc],
                                     in1=n_ps[:, 0:wc])
                nact = sb2.tile([HID, 512], f16, tag="nact")
                nc.scalar.activation(out=nact[:, 0:wc], in_=pre[:, 0:wc],
                                     func=AF.Tanh, bias=bihn_t[0:HID, :])

                h_sb = sb2.tile([HID, 512], f16, tag="h_sb")
                if HZERO:
                    zn = sb2.tile([HID, 512], f16, tag="zn")
                    nc.vector.tensor_mul(out=zn[:, 0:wc], in0=z_sb[:, 0:wc],
                                         in1=nact[:, 0:wc])
                    nc.vector.tensor_sub(out=h_sb[:, 0:wc], in0=nact[:, 0:wc],
                                         in1=zn[:, 0:wc])
                else:
                    d_sb = sb2.tile([HID, 512], f16, tag="d_sb")
                    nc.vector.tensor_sub(out=d_sb[:, 0:wc],
                                         in0=hmemT_t[0:HID, c0:c0 + wc],
                                         in1=nact[:, 0:wc])
                    e_sb = sb2.tile([HID, 512], f16, tag="e_sb")
                    nc.vector.tensor_mul(out=e_sb[:, 0:wc], in0=z_sb[:, 0:wc],
                                         in1=d_sb[:, 0:wc])
                    nc.vector.tensor_add(out=h_sb[:, 0:wc], in0=nact[:, 0:wc],
                                         in1=e_sb[:, 0:wc])
                nc.sync.dma_start(out=out_d[:, c0:c0 + wc], in_=h_sb[:, 0:wc])

            do_group(0)
            for t, d in loads:
                nc.sync.dma_start(out=t[:], in_=d[:])
            blocks_done = GS[0]
            pair_next = 0
            for gi in range(1, len(GS)):
                do_group(gi)
                blocks_done += GS[gi]
                if 'phase2' in skip:
                    continue
                while pair_next < NCH and blocks_done >= min(4 * pair_next + 4 + PAIR_LAG, BLOCKS):
                    do_chunk(pair_next)
                    pair_next += 1
            if 'phase2' not in skip:
                while pair_next < NCH:
                    do_chunk(pair_next)
                    pair_next += 1

    return nc


def kernel(x, edge_index, edge_weight, W_gcn, b_gcn, W_ih, W_hh, b_ih, b_hh, h_mem):
    global last_nc
    import concourse.mybir as mybir
    from concourse.bass_utils import run_bass_kernel_spmd

    x = np.asarray(x, dtype=np.float32)
    h_mem = np.asarray(h_mem, dtype=np.float32)
    W_gcn = np.asarray(W_gcn, dtype=np.float32)
    W_ih = np.asarray(W_ih, dtype=np.float32)
    W_hh = np.asarray(W_hh, dtype=np.float32)
    b_gcn = np.asarray(b_gcn, dtype=np.float32)
    b_ih = np.asarray(b_ih, dtype=np.float32)
    b_hh = np.asarray(b_hh, dtype=np.float32)

    src = np.asarray(edge_index[0], dtype=np.int64)
    dst = np.asarray(edge_index[1], dtype=np.int64)
    w = np.asarray(edge_weight, dtype=np.float32)
    deg = (np.bincount(dst, weights=w.astype(np.float64), minlength=N)
           .astype(np.float32) + np.float32(1.0))
    dinv2 = (np.float32(1.0) / deg).astype(np.float32)

    pl, idx_w, dstnT, normeT, normoT, dstF, normFe, normFo = _host_prep(
        x, edge_index, edge_weight)

    hzero = not np.any(h_mem)
    nc = _build_program(pl, hzero=hzero)
    last_nc = nc

    mybir.codegen_inst_isa_subclasses(nc)
    _split_sync_waits(nc)

    x16 = x.astype(np.float16)
    x16_pairs = np.ascontiguousarray(x16.reshape(N // 2, 2 * IN_C))

    # selfx[c][p, b*128+f] = dinv2[v] * x[v, f], v = node_at[c, b*128+p]
    selfx = np.zeros((CORES, P, SHARDP), np.float16)
    na = pl.node_at                              # [CORES, SHARDP]
    for c in range(CORES):
        nav = na[c].reshape(BLOCKS, P)
        for b in range(BLOCKS):
            vs = nav[b]
            ok = vs >= 0
            rows = np.zeros((P, IN_C), np.float32)
            rows[ok] = x[vs[ok]] * dinv2[vs[ok]][:, None]
            selfx[c, :, b * P:(b + 1) * P] = rows.astype(np.float16)

    ident = np.eye(P, dtype=np.float16)
    iota_np = np.broadcast_to(np.arange(P, dtype=np.float16), (P, P)).copy()

    b_ihp = (b_ih + W_ih @ b_gcn).astype(np.float32)
    br = np.tile((b_ihp[0:HID] + b_hh[0:HID]).astype(np.float32), 2).reshape(P, 1)
    bz = np.tile((b_ihp[HID:2 * HID] + b_hh[HID:2 * HID]).astype(np.float32), 2).reshape(P, 1)
    bihn = np.tile(b_ihp[2 * HID:3 * HID].astype(np.float32), 2).reshape(P, 1)
    bhhn = np.tile(b_hh[2 * HID:3 * HID].astype(np.float32), 2).reshape(P, 1)

    wgcnT = np.ascontiguousarray(W_gcn.T).astype(np.float16)
    wihT = np.ascontiguousarray(np.vstack([W_ih.T, W_ih.T])).astype(np.float16)
    whhT = np.ascontiguousarray(np.vstack([W_hh.T, W_hh.T])).astype(np.float16)

    hmemT = np.zeros((CORES, HID, SHARDP), np.float16)
    if np.any(h_mem):
        for c in range(CORES):
            ok = na[c] >= 0
            hmemT[c][:, ok] = h_mem[na[c][ok]].T.astype(np.float16)

    in_maps = []
    for c in range(CORES):
        in_maps.append({
            "x16": x16_pairs, "idx": idx_w[c], "dstn": dstnT[c],
            "norme": normeT[c], "normo": normoT[c],
            "dstf": dstF[c], "normfe": normFe[c], "normfo": normFo[c],
            "selfx": selfx[c], "ident": ident, "iota": iota_np,
            "wgcnT": wgcnT, "wihT": wihT, "whhT": whhT,
            "br": br, "bz": bz, "bihn": bihn, "bhhn": bhhn, "hmemT": hmemT[c],
        })

    res = run_bass_kernel_spmd(nc, in_maps, core_ids=list(range(CORES)))
    out = np.empty((N, HID), np.float32)
    for c in range(CORES):
        o = res.results[c]["outT"].astype(np.float32)    # [HID, SHARDP]
        ok = na[c] >= 0
        out[na[c][ok]] = o[:, ok].T
    return out
